# revision 30
# baseline (speedup 1.0000x reference)
"""CBOW negative-sampling loss kernel for 8 Trainium2 NeuronCores.

The reference computes one-hot @ table matmuls (embedding lookups in
disguise) followed by a tiny log-sigmoid loss.  Device-side algorithm:

Phase A (index extraction, streaming):
  Every one-hot row (50000 wide) is laid out as 4 partitions x 12500.
  The iota tile (value 65536 + (p%4)*12500 + j) is generated on-device
  (gpsimd iota + one DVE pass) instead of being DMAed from DRAM, so the
  HBM stream is exactly the one-hot bytes.  Each [128, 6250] chunk is
  consumed by ONE fused DVE tensor_tensor_reduce (mult + row-sum), and
  a per-quarter [128]->[128] fold matmul places each tile's 32 row
  values directly into a [128, col] PSUM layout (flat row = 128*col+p),
  eliminating the DRAM scratch round trips of the earlier version.

Phase B (gather + dots, pipelined per column):
  vo is tile 0 and owns psum column 0: its indices are ready early, so
  the 32 V rows are gathered and replicated to all 128 row slots of
  each column with tiny 0/1 matmuls on the (otherwise idle) tensor
  engine while streaming continues.  Each of the 4 vi/neg columns is
  finished as soon as its 4 extraction tiles land: idx = val - 65536*cnt,
  indirect-gather 128 U rows, fused DVE dot against the replicated V
  rows.  Only the last column's ~10us of work trails the final DMA.

Host: batch-shard across 8 cores, log-sigmoid loss terms + mean on CPU.
"""
import numpy as np

import concourse.bass as bass
import concourse.mybir as mybir
from concourse.tile import TileContext
from concourse.bass_utils import run_bass_kernel_spmd

VOC = 50000
EMB = 300
B = 256
CTX = 6
K = 10
NCORES = 8
BPC = B // NCORES                    # 32 batch rows per core
NV = BPC * CTX                       # 192 vi rows per core
NN = BPC * K                         # 320 neg rows per core
NTILES = 1 + (NV + NN) // 32         # 17 extraction tiles of [128, 12500]
QW = VOC // 4                        # 12500 per partition-quarter
CH = QW // 2                         # 6250 free-dim chunk
COLS = [4, 4, 4, 3, 1]               # vi+neg tiles per psum column
NCOL = len(COLS)                     # 5 columns: 128,128,128,96,32 rows
CVALID = [32 * n for n in COLS]      # valid partitions per column
MARK = 65536.0                       # cnt marker (> max idx, power of 2)
# last extraction tile streams as small chunks to shorten the serial tail
LAST_CHUNKS = [(0, 3125), (3125, 3125), (6250, 3125), (9375, 1563),
               (10938, 781), (11719, 781)]
# out_t column layout [d1 d2 d3 | cnt1..cnt4 | d4 d5 cnt5]: everything in
# cols 0..6 is complete when column 4 closes (ships mid-stream in one DMA);
# cols 7..9 complete in the tail (one final DMA).
DCOL = [0, 1, 2, 7, 8]               # d column of out_t, per psum column
CCOL = [3, 4, 5, 6, 9]               # cnt column of out_t, per psum column

F32 = mybir.dt.float32
I32 = mybir.dt.int32


def _split_multi_waits(nc):
    """This env's walrus accepts only ONE sync wait per instruction.
    Hoist extra waits into single-wait NoOps right before the owner."""
    cnt = 0
    for fn in nc.m.functions:
        for blk in fn.blocks:
            insts = list(blk.instructions)
            if not any(
                i.sync_info and i.sync_info.on_wait and len(i.sync_info.on_wait) > 1
                for i in insts
            ):
                continue
            new = []
            for inst in insts:
                si = inst.sync_info
                if si and si.on_wait and len(si.on_wait) > 1:
                    waits = list(si.on_wait)
                    for w in waits[:-1]:
                        cnt += 1
                        nop = mybir.InstNoOp(
                            name=f"mwsplit-{cnt}", engine=inst.engine, ins=[], outs=[]
                        )
                        nop.sync_info = mybir.SyncInfo(on_wait=[w], on_update=[])
                        new.append(nop)
                    inst.sync_info = mybir.SyncInfo(
                        on_wait=[waits[-1]], on_update=list(si.on_update or [])
                    )
                new.append(inst)
            blk.instructions = new
    return cnt


def _build(split_waits=True):
    nc = bass.Bass(enable_partition_id=False)

    vo = nc.declare_dram_parameter("vo", [BPC, VOC], F32, isOutput=False)
    vi = nc.declare_dram_parameter("vi", [NV, VOC], F32, isOutput=False)
    ng = nc.declare_dram_parameter("ng", [NN, VOC], F32, isOutput=False)
    V = nc.declare_dram_parameter("V", [VOC, EMB], F32, isOutput=False)
    U = nc.declare_dram_parameter("U", [VOC, EMB], F32, isOutput=False)
    qoff = nc.declare_dram_parameter("qoff", [128, 1], F32, isOutput=False)
    foldq = nc.declare_dram_parameter("foldq", [128, 4 * 128], F32, isOutput=False)
    repmat = nc.declare_dram_parameter("repmat", [32, NCOL * 128], F32, isOutput=False)
    d_out = nc.declare_dram_parameter("out", [128, 2 * NCOL], F32, isOutput=True)

    # per-tile [128, QW] sources: 4 partition-quarters per row.
    # tile 0 = vo (owns psum column 0); tiles 1..16 = vi(6) + neg(10),
    # grouped 4 tiles -> one 128-row column (flat row f = 128*(c-1) + p).
    srcs = [vo.rearrange("r (q f) -> (r q) f", q=4)]
    for u in range(CTX):
        srcs.append(vi[32 * u:32 * (u + 1), :].rearrange("r (q f) -> (r q) f", q=4))
    for u in range(K):
        srcs.append(ng[32 * u:32 * (u + 1), :].rearrange("r (q f) -> (r q) f", q=4))
    assert len(srcs) == NTILES

    with TileContext(nc) as tc:
        with (
            tc.tile_pool(name="const", bufs=1) as cpool,
            tc.tile_pool(name="data", bufs=5) as dpool,
            tc.tile_pool(name="vals", bufs=3) as vpool,
            tc.tile_pool(name="small", bufs=1) as spool,
            tc.tile_pool(name="col", bufs=2) as lpool,
            tc.tile_pool(name="gath", bufs=3) as gpool,
            tc.tile_pool(name="pcol", bufs=2, space="PSUM") as pcpool,
            tc.tile_pool(name="repp", bufs=2, space="PSUM") as rpool,
            tc.tile_pool(name="vops", bufs=1, space="PSUM") as opool,
        ):
            # ------- constants: ACT HWDGE ring, keeps SP ring streaming ----
            qoff_t = cpool.tile([128, 1], F32, tag="qoff")
            nc.scalar.dma_start(out=qoff_t[:], in_=qoff[:])
            foldq_t = cpool.tile([128, 4 * 128], F32, tag="foldq")
            nc.scalar.dma_start(out=foldq_t[:], in_=foldq[:])
            repmat_t = cpool.tile([32, NCOL * 128], F32, tag="repmat")
            nc.scalar.dma_start(out=repmat_t[:], in_=repmat[:])

            # iota[p, j] = 65536 + (p%4)*12500 + j, generated in halves so
            # the first chunk's DVE op can start ~15us sooner.
            iota_t = cpool.tile([128, QW], F32, tag="iota")
            for ih in range(2):
                sl = slice(ih * CH, (ih + 1) * CH)
                nc.gpsimd.iota(
                    out=iota_t[:, sl], pattern=[[1, CH]], base=int(MARK) + ih * CH,
                    channel_multiplier=0, allow_small_or_imprecise_dtypes=True,
                )
                nc.vector.tensor_scalar(
                    out=iota_t[:, sl], in0=iota_t[:, sl], scalar1=qoff_t[:, 0:1],
                    scalar2=None, op0=mybir.AluOpType.add,
                )

            out_t = spool.tile([128, 2 * NCOL], F32, tag="out_t")
            nc.vector.memset(out_t[:], 0.0)
            repVs = [None] * (NCOL + 1)

            # ---------------- streaming extraction + pipelined columns -----
            # The fold halves accumulate straight into a [128, 1] PSUM column
            # (chained tiny matmuls per column), so no DVE reduce is needed.
            # The last column is a single 32-row tile and the last tile
            # streams as small chunks, so the post-stream serial tail is a
            # 32-row gather + dot only.
            col_of_tile, m_of_tile, starts = [], [], []
            s = 1
            for ci, n in enumerate(COLS):
                starts.append(s)
                for mm in range(n):
                    col_of_tile.append(ci + 1)
                    m_of_tile.append(mm)
                s += n
            pcol = None
            pending = []              # (column, rowU tile) awaiting their dot
            for t in range(NTILES):
                if t == NTILES - 1:
                    chunks = LAST_CHUNKS
                else:
                    chunks = [(0, CH), (CH, CH)]
                if t == 0:
                    c, m, ntile = 0, 0, 1
                    pcol = opool.tile([128, 1], F32, tag="pvo")
                else:
                    c, m = col_of_tile[t - 1], m_of_tile[t - 1]
                    ntile = COLS[c - 1]
                    if m == 0:
                        pcol = pcpool.tile([128, 1], F32, tag="pcol")
                vt = vpool.tile([128, len(LAST_CHUNKS)], F32, tag="vt")
                for h, (off, csz) in enumerate(chunks):
                    chunk = dpool.tile([128, CH], F32, tag="chunk")
                    nc.sync.dma_start(
                        out=chunk[:, :csz], in_=srcs[t][:, off:off + csz]
                    )
                    # fused: prod = (chunk * 1) * iota ; vt[:,h] = sum(prod)
                    nc.vector.scalar_tensor_tensor(
                        out=chunk[:, :csz], in0=chunk[:, :csz], scalar=1.0,
                        in1=iota_t[:, off:off + csz],
                        op0=mybir.AluOpType.mult, op1=mybir.AluOpType.mult,
                        accum_out=vt[:, h:h + 1],
                    )
                    nc.tensor.matmul(
                        out=pcol[:], lhsT=foldq_t[:, 128 * m:128 * (m + 1)],
                        rhs=vt[:, h:h + 1],
                        start=(m == 0 and h == 0),
                        stop=(m == ntile - 1 and h == len(chunks) - 1),
                    )

                if t == 0:
                    # vo rows always valid: ofs = val - MARK (i32 cast out)
                    ofsv = spool.tile([32, 1], I32, tag="ofsv")
                    nc.vector.tensor_scalar(
                        out=ofsv[:], in0=pcol[0:32, :], scalar1=-MARK,
                        scalar2=None, op0=mybir.AluOpType.add,
                    )
                    voV = spool.tile([32, EMB], F32, tag="voV")
                    nc.gpsimd.indirect_dma_start(
                        out=voV[:], out_offset=None, in_=V[:],
                        in_offset=bass.IndirectOffsetOnAxis(ap=ofsv[:], axis=0),
                    )
                    continue
                if m != ntile - 1:
                    continue

                # -------- column c complete: extract idx, gather, dot ------
                vc = CVALID[c - 1]
                cc_ = CCOL[c - 1]
                cnt = out_t[:, cc_:cc_ + 1]
                nc.vector.tensor_scalar(
                    out=cnt, in0=pcol[:], scalar1=MARK, scalar2=None,
                    op0=mybir.AluOpType.is_ge,
                )
                ofsc = lpool.tile([128, 1], I32, tag="ofsc")
                nc.vector.scalar_tensor_tensor(
                    out=ofsc[:], in0=cnt, scalar=-MARK, in1=pcol[:],
                    op0=mybir.AluOpType.mult, op1=mybir.AluOpType.add,
                )
                if c == 1:
                    # replicate V[vo[b]] to every flat row slot per column;
                    # emitted here (after column 1's PSUM group closed) so
                    # accumulation groups never interleave on PE.  Each repV
                    # is copied to SBUF: the gpsimd dot cannot read PSUM.
                    for cc in range(1, NCOL + 1):
                        repP = rpool.tile([128, EMB], F32, tag="repP")
                        nc.tensor.matmul(
                            out=repP[:],
                            lhsT=repmat_t[:, 128 * (cc - 1):128 * cc],
                            rhs=voV[:], start=True, stop=True,
                        )
                        repV = cpool.tile([128, EMB], F32, tag=f"repV{cc}")
                        nc.scalar.activation(
                            out=repV[:], in_=repP[:],
                            func=mybir.ActivationFunctionType.Copy,
                        )
                        repVs[cc] = repV
                rowU = gpool.tile([128, EMB], F32, tag="rowU")
                nc.gpsimd.indirect_dma_start(
                    out=rowU[:vc, :], out_offset=None, in_=U[:],
                    in_offset=bass.IndirectOffsetOnAxis(ap=ofsc[:vc, :], axis=0),
                )
                # The fused dot d = sum(U_row * V_vo_row) runs on DVE, but
                # DEFERRED one column: column c's dot is emitted at column
                # c+1's close, ~70us after its gather landed, so the
                # in-order DVE stream never idles waiting for a gather.
                pending.append((c, rowU))
                if len(pending) > 1:
                    pc, prowU = pending.pop(0)
                    pvc, pdc = CVALID[pc - 1], DCOL[pc - 1]
                    # gate = (pcol >= -1) == all-ones, but DEPENDS on the
                    # current column's fold: the Tile scheduler provably
                    # cannot hoist the dot ahead of this point, so its
                    # gather has had a full column (~70us) to land.
                    gate = lpool.tile([128, 1], F32, tag="gate")
                    nc.vector.tensor_scalar(
                        out=gate[:], in0=pcol[:], scalar1=-1.0, scalar2=None,
                        op0=mybir.AluOpType.is_ge,
                    )
                    nc.vector.scalar_tensor_tensor(
                        out=prowU[:pvc, :], in0=prowU[:pvc, :],
                        scalar=gate[:pvc, :], in1=repVs[pc][:pvc, :],
                        op0=mybir.AluOpType.mult, op1=mybir.AluOpType.mult,
                        accum_out=out_t[:pvc, pdc:pdc + 1],
                    )
                if c == NCOL - 1:
                    # d1..d3 + cnt1..cnt4 ship while the last tile streams;
                    # ACT ring, so the SP chunk-DMA ring never waits on it.
                    nc.scalar.dma_start(out=d_out[:, 0:7], in_=out_t[:, 0:7])

            # final dots (columns 4 and 5) + the last three output columns
            for pc, prowU in pending:
                pvc, pdc = CVALID[pc - 1], DCOL[pc - 1]
                nc.vector.scalar_tensor_tensor(
                    out=prowU[:pvc, :], in0=prowU[:pvc, :], scalar=1.0,
                    in1=repVs[pc][:pvc, :],
                    op0=mybir.AluOpType.mult, op1=mybir.AluOpType.mult,
                    accum_out=out_t[:pvc, pdc:pdc + 1],
                )
            nc.scalar.dma_start(out=d_out[:, 7:10], in_=out_t[:, 7:10])

    if split_waits:
        _split_multi_waits(nc)
    return nc


def _col_starts():
    st, s = [], 0
    for n in COLS:
        st.append(s)
        s += 32 * n
    return st


def _consts():
    p = np.arange(128)
    qoff_np = ((p % 4) * QW).astype(np.float32).reshape(128, 1)
    foldq_np = np.zeros((128, 4 * 128), np.float32)
    for m in range(4):
        foldq_np[p, 128 * m + 32 * m + p // 4] = 1.0
    repmat_np = np.zeros((32, NCOL * 128), np.float32)
    starts = _col_starts()
    for c in range(NCOL):
        for pp in range(CVALID[c]):
            f = starts[c] + pp
            b = f // CTX if f < NV else (f - NV) // K
            repmat_np[b, 128 * c + pp] = 1.0
    return qoff_np, foldq_np, repmat_np


_CACHE = {}


def kernel(vo, vi, neg_samples, V, U):
    if "nc" not in _CACHE:
        _CACHE["nc"] = _build()
        _CACHE["consts"] = _consts()
    nc = _CACHE["nc"]
    qoff_np, foldq_np, repmat_np = _CACHE["consts"]

    vo = np.ascontiguousarray(vo, dtype=np.float32)
    vi = np.ascontiguousarray(vi, dtype=np.float32)
    neg = np.ascontiguousarray(neg_samples, dtype=np.float32)
    V = np.ascontiguousarray(V, dtype=np.float32)
    U = np.ascontiguousarray(U, dtype=np.float32)

    in_maps = []
    for c in range(NCORES):
        sl = slice(c * BPC, (c + 1) * BPC)
        in_maps.append({
            "vo": vo[sl],
            "vi": vi[sl].reshape(NV, VOC),
            "ng": neg[sl].reshape(NN, VOC),
            "V": V,
            "U": U,
            "qoff": qoff_np,
            "foldq": foldq_np,
            "repmat": repmat_np,
        })

    res = run_bass_kernel_spmd(nc, in_maps, list(range(NCORES)))
    obs = []
    for r in res.results:
        o = r["out"]
        d_flat = np.concatenate([o[:CVALID[c], DCOL[c]] for c in range(NCOL)])
        c_flat = np.concatenate([o[:CVALID[c], CCOL[c]] for c in range(NCOL)])
        d_vi = d_flat[:NV].reshape(BPC, CTX)
        c_vi = c_flat[:NV].reshape(BPC, CTX)
        d_ng = d_flat[NV:NV + NN].reshape(BPC, K)
        lp = (d_vi * c_vi).sum(axis=1)
        ms = c_vi.sum(axis=1)
        x = lp / ms
        left = -np.log1p(np.exp(-x))
        right = (-np.log1p(np.exp(d_ng))).sum(axis=1)
        obs.append(-(left + right))
    ob = np.concatenate(obs)
    return np.float32(ob.mean(dtype=np.float64))
